# revision 1
# baseline (speedup 1.0000x reference)
"""Trainium2 Bass kernel for nn_Attention1D (B=4, L=4096, C=64).

reference:
    Q = x@Wq + bq ; K = x@Wk + bk ; V = x@Wv + bv          (per batch b)
    s = Q @ K.T / sqrt(C)                                   [L_q, L_k]
    attn = softmax(s, axis=q)      # normalize over QUERY axis
    out = attn @ V + x

Sharding: 8 cores = 4 batches x 2 key-shards (k in [0,2048) / [2048,4096)).
The softmax normalizes over q, which is NOT sharded, so each core's softmax
is fully local:
    Z[k]   = sum_q exp(s[q,k])
    out_qf = sum_k exp(s[q,k]) * (V[k,f]/Z[k])
and the two k-shards' partial outputs simply ADD. The host sums the pair
and adds the residual x (the residual dominates the output, which also
makes the attention path tolerant of bf16).

Layout: channel-major (c on partitions) everywhere, so scores come out
transposed sT[k, q] with the softmax axis on the free dim:
    sT chunk = matmul(lhsT=KT[c,k-tile(128)], rhs=QT[c,q-chunk(512)])  f32r
    exp+Z    = one ScalarE pass per [128,1024] PSUM chunk (accum_out)
    out      = PSUM-accumulated over 16 k-tiles:
               matmul(acc[qc], lhsT=ET[k,qc*128:+128](bf16), rhs=GV[k,f])
Host pre-transposes x and appends a ones-row so biases ride inside the
weights (contract dim 65); 1/sqrt(C) is folded into Wq. Q/K path runs in
float32r (fp32 data, full-rate PE mode, ~tf32 precision) because softmax
exponentiates absolute score errors: bf16 Q/K costs 1e-2 rel error, f32r
1e-4. V/ET stay bf16. No max-subtraction (|s| <= ~9, exp is safe in fp32).

PSUM (8 banks): 2 x [128,1024]f32 score slots (4 banks, double-buffered,
evacuated directly by the ACT exp) + [128,32,64]f32 out accumulator
(4 banks). matmul start=True clears has_written for the WHOLE bank, so only
the first accumulator chunk-MM per bank sets it.

A ~7us dummy-matmul warmup burst runs during the input DMAs: the PE's HAM
clock gate only reaches 2.4 GHz after ~3.4us of *continuous* busy; without
it the whole kernel runs at 1.2 GHz.
"""

import numpy as np
import ml_dtypes  # noqa: F401  (np bf16 support registered on import)

B, L, C = 4, 4096, 64
NCORES = 8
KSH = L // 2          # k columns per core: 2048
NKT = KSH // 128      # 16 k-tiles per core
NQC = L // 128        # 32 q-chunks of 128
NQ5 = L // 512        # 8 q-chunks of 512

_cache = {}


def _patch_ldw_opt():
    # walrus is invoked with --enable-ldw-opt=false hardcoded; redundant
    # LDWEIGHTS (8 same-weight score matmuls per k-tile) cost ~40us/core.
    import concourse.bass_utils as bu
    if getattr(bu, "_ldw_patched", False):
        return
    orig = bu.run_command

    def run_command_ldw(cmd, *a, **kw):
        if isinstance(cmd, list):
            cmd = [c.replace("--enable-ldw-opt=false", "--enable-ldw-opt=true")
                   if isinstance(c, str) else c for c in cmd]
        return orig(cmd, *a, **kw)

    bu.run_command = run_command_ldw
    bu._ldw_patched = True


def _build():
    import concourse.bacc as bacc
    import concourse.mybir as mybir
    import concourse.tile as tile
    from concourse.bass import _add_dep_helper


    bf16 = mybir.dt.bfloat16
    f32 = mybir.dt.float32
    f32r = mybir.dt.float32r
    i32 = mybir.dt.int32
    AF = mybir.ActivationFunctionType
    AX = mybir.AxisListType

    nc = bacc.Bacc("TRN2", target_bir_lowering=False, debug=False)

    xt_d = nc.dram_tensor("xt", [C + 1, L], f32r, kind="ExternalInput")
    xk_d = nc.dram_tensor("xk", [C + 1, KSH], f32r, kind="ExternalInput")
    wq_d = nc.dram_tensor("wq", [C + 1, 2 * C], f32r, kind="ExternalInput")
    wk_d = nc.dram_tensor("wk", [C + 1, 2 * C], f32r, kind="ExternalInput")
    wv_d = nc.dram_tensor("wv", [C + 1, C], f32r, kind="ExternalInput")
    o_d = nc.dram_tensor("o", [L, C], f32, kind="ExternalOutput")

    with tile.TileContext(nc) as tc:
        with (
            tc.tile_pool(name="consts", bufs=1) as consts,
            tc.tile_pool(name="sb", bufs=1) as sb,
            tc.tile_pool(name="etp", bufs=4) as etp,
            tc.tile_pool(name="gvp", bufs=4) as gvp,
            tc.tile_pool(name="zpp", bufs=6) as zpp,
            tc.tile_pool(name="scp", bufs=2, space="PSUM") as scp,
            tc.tile_pool(name="accp", bufs=1, space="PSUM") as accp,
        ):
            # --- HAM warmup: dense dummy matmuls while the DMAs stream in ---
            wu = consts.tile([128, 512], bf16)
            nc.vector.memset(wu, 0.0)
            for _ in range(10):
                ps = scp.tile([128, 512], f32, tag="s")
                nc.tensor.matmul(ps, lhsT=wu[:, 0:128], rhs=wu,
                                 start=True, stop=True)

            wq_s = consts.tile([C + 1, 2 * C], f32r)
            wk_s = consts.tile([C + 1, 2 * C], f32r)
            wv_s = consts.tile([C + 1, C], f32r)
            nc.sync.dma_start(out=wq_s, in_=wq_d.ap())
            nc.sync.dma_start(out=wk_s, in_=wk_d.ap())
            nc.sync.dma_start(out=wv_s, in_=wv_d.ap())

            # per-512-chunk input tiles -> precise DMA->matmul dependencies
            xt_c = []
            for c in range(NQ5):
                t = sb.tile([C + 1, 512], f32r, tag=f"xt{c}")
                nc.sync.dma_start(out=t, in_=xt_d.ap()[:, c * 512:(c + 1) * 512])
                xt_c.append(t)
            xk_c = []
            for c in range(KSH // 512):
                t = sb.tile([C + 1, 512], f32r, tag=f"xk{c}")
                nc.sync.dma_start(out=t, in_=xk_d.ap()[:, c * 512:(c + 1) * 512])
                xk_c.append(t)

            # QT/KT chunks [128, 512]: rows 0-63 and 64-127 hold the SAME
            # values (weights doubled host-side) so score matmuls can be
            # row-packed two k-tiles at a time via tile_position.
            qt_c = []
            for c in range(NQ5):
                ps = scp.tile([128, 512], f32, tag="s")
                nc.tensor.matmul(ps, lhsT=wq_s, rhs=xt_c[c],
                                 start=True, stop=True)
                t = sb.tile([128, 512], f32r, tag=f"qt{c}")
                nc.vector.tensor_copy(out=t, in_=ps)
                qt_c.append(t)
            kt_c = []
            for c in range(KSH // 512):
                ps = scp.tile([128, 512], f32, tag="s")
                nc.tensor.matmul(ps, lhsT=wk_s, rhs=xk_c[c],
                                 start=True, stop=True)
                t = sb.tile([128, 512], f32r, tag=f"kt{c}")
                nc.vector.tensor_copy(out=t, in_=ps)
                kt_c.append(t)

            v_ts = []  # V [k(128), f] per k-tile, bf16
            for kt in range(NKT):
                vps = scp.tile([128, C], f32, tag="s")
                nc.tensor.matmul(
                    vps,
                    lhsT=xk_c[kt // 4][:, (kt % 4) * 128:(kt % 4 + 1) * 128],
                    rhs=wv_s, start=True, stop=True,
                )
                v_t = sb.tile([128, C], bf16, tag=f"v{kt}")
                nc.vector.tensor_copy(out=v_t, in_=vps)
                v_ts.append(v_t)

            # --- main loop over k-tiles ---
            acc = accp.tile([128, NQC, C], f32)   # 4 PSUM banks, whole loop
            prev = None

            def emit_av_group(p, c2):
                # 8 AV chunk-MMs of the previous k-tile, interleaved between
                # score chunks to keep the PE dense.
                et_p, gv_p, kt_p = p
                for qc in range(c2 * 8, c2 * 8 + 8):
                    # start=True clears has_written for the WHOLE bank: only
                    # the first chunk-MM per bank may set it; later chunks
                    # overwrite-where-unset, which sets their own bits.
                    nc.tensor.matmul(
                        acc[:, qc, :],
                        lhsT=et_p[:, qc * 128:(qc + 1) * 128],
                        rhs=gv_p,
                        start=(kt_p == 0 and qc % 8 == 0),
                        stop=(kt_p == NKT - 1),
                        skip_group_check=True,
                    )

            # k-tiles processed in PAIRS: the score matmuls contract only 64
            # channels, so tile A runs in PE rows 0-63 and tile B in rows
            # 64-127 concurrently (tile_position row packing) -> ~2x.
            for kp in range(NKT // 2):
                kA, kB = 2 * kp, 2 * kp + 1
                etA = etp.tile([128, L], bf16, tag="etA")
                etB = etp.tile([128, L], bf16, tag="etB")
                zpA = zpp.tile([128, 4], f32, tag="zpA")
                zpB = zpp.tile([128, 4], f32, tag="zpB")
                lA = kt_c[kA // 4][0:C, (kA % 4) * 128:(kA % 4 + 1) * 128]
                lB = kt_c[kB // 4][C:128, (kB % 4) * 128:(kB % 4 + 1) * 128]
                for c2 in range(4):
                    stA = scp.tile([128, 1024], f32, tag="s")
                    stB = scp.tile([128, 1024], f32, tag="s")
                    last = None
                    for h in range(2):
                        rhs = qt_c[c2 * 2 + h]
                        ma = nc.tensor.matmul(
                            stA[:, h * 512:(h + 1) * 512], lhsT=lA,
                            rhs=rhs[0:C, :], tile_position=(0, 0),
                            start=True, stop=True,
                        )
                        mb = nc.tensor.matmul(
                            stB[:, h * 512:(h + 1) * 512], lhsT=lB,
                            rhs=rhs[C:128, :], tile_position=(C, 0),
                            start=True, stop=True,
                        )
                        # keep the A/B pair adjacent in the static PE order so
                        # the row-packed halves co-issue (scheduler otherwise
                        # sometimes emits [B,B,A,A], serializing the pair)
                        if last is not None:
                            _add_dep_helper(ma.ins, last.ins, sync=False,
                                            reason="pair order")
                        _add_dep_helper(mb.ins, ma.ins, sync=False,
                                        reason="pair order")
                        last = mb
                    nc.scalar.activation(
                        out=etA[:, c2 * 1024:(c2 + 1) * 1024], in_=stA,
                        func=AF.Exp, accum_out=zpA[:, c2:c2 + 1],
                    )
                    nc.scalar.activation(
                        out=etB[:, c2 * 1024:(c2 + 1) * 1024], in_=stB,
                        func=AF.Exp, accum_out=zpB[:, c2:c2 + 1],
                    )
                    if prev is not None:
                        emit_av_group(prev[0], c2)
                        emit_av_group(prev[1], c2)
                gvs = []
                for kt, zp, vv in ((kA, zpA, v_ts[kA]), (kB, zpB, v_ts[kB])):
                    z = zpp.tile([128, 1], f32, tag=f"z{kt % 2}")
                    nc.vector.reduce_sum(out=z, in_=zp, axis=AX.X)
                    rz = zpp.tile([128, 1], f32, tag=f"rz{kt % 2}")
                    nc.vector.reciprocal(out=rz, in_=z)
                    gv = gvp.tile([128, C], bf16, tag=f"gv{kt % 2}")
                    nc.vector.tensor_scalar_mul(gv, vv, rz)
                    gvs.append(gv)
                prev = ((etA, gvs[0], kA), (etB, gvs[1], kB))
            # final pair's AV drain, interleaved with the per-bank
            # evacuation + store so the tail overlaps the remaining AV work
            o_ap = o_d.ap()
            for g in range(4):
                emit_av_group(prev[0], g)
                emit_av_group(prev[1], g)
                ob = sb.tile([128, 8, C], f32, tag=f"ob{g}")
                nc.vector.tensor_copy(out=ob, in_=acc[:, g * 8:(g + 1) * 8, :])
                nc.sync.dma_start(
                    out=o_ap[g * 1024:(g + 1) * 1024, :].rearrange(
                        "(t p) f -> p t f", p=128
                    ),
                    in_=ob,
                )

    nc.compile()
    return nc


def _get_nc():
    if "nc" not in _cache:
        _cache["nc"] = _build()
    return _cache["nc"]


def _in_maps(x, Wq, bq, Wk, bk, Wv, bv):
    s = 1.0 / np.sqrt(np.float32(C))
    wq1 = (np.concatenate([Wq, bq[None, :]], 0) * s).astype(np.float32)
    wq1 = np.concatenate([wq1, wq1], 1)          # doubled -> replicated QT
    wk1 = np.concatenate([Wk, bk[None, :]], 0).astype(np.float32)
    wk1 = np.concatenate([wk1, wk1], 1)
    wv1 = np.concatenate([Wv, bv[None, :]], 0).astype(np.float32)
    maps = []
    for core in range(NCORES):
        b, half = core // 2, core % 2
        x1t = np.ascontiguousarray(np.concatenate(
            [x[b], np.ones((L, 1), np.float32)], 1
        ).T.astype(np.float32))              # [65, L]
        xk = np.ascontiguousarray(x1t[:, half * KSH:(half + 1) * KSH])
        maps.append({
            "xt": x1t,
            "xk": xk,
            "wq": wq1, "wk": wk1, "wv": wv1,
        })
    return maps


def _run(x, Wq, bq, Wk, bk, Wv, bv, trace=False):
    from concourse.bass_utils import run_bass_kernel_spmd

    nc = _get_nc()
    maps = _in_maps(x, Wq, bq, Wk, bk, Wv, bv)
    res = run_bass_kernel_spmd(
        nc, maps, core_ids=list(range(NCORES)), trace=trace
    )
    outs = [r["o"].astype(np.float32) for r in res.results]
    full = np.empty((B, L, C), np.float32)
    for b in range(B):
        full[b] = outs[2 * b] + outs[2 * b + 1] + x[b]
    return full, res


def kernel(x, Wq, bq, Wk, bk, Wv, bv):
    x = np.asarray(x, np.float32)
    full, _ = _run(
        x,
        np.asarray(Wq, np.float32), np.asarray(bq, np.float32),
        np.asarray(Wk, np.float32), np.asarray(bk, np.float32),
        np.asarray(Wv, np.float32), np.asarray(bv, np.float32),
    )
    return full



# revision 5
# speedup vs baseline: 1.1209x; 1.1209x over previous
"""Trainium2 Bass kernel for nn_Attention1D (B=4, L=4096, C=64).

reference:
    Q = x@Wq + bq ; K = x@Wk + bk ; V = x@Wv + bv          (per batch b)
    s = Q @ K.T / sqrt(C)                                   [L_q, L_k]
    attn = softmax(s, axis=q)      # normalize over QUERY axis
    out = attn @ V + x

Sharding: 8 cores = 4 batches x 2 key-shards (k in [0,2048) / [2048,4096)).
The softmax normalizes over q, which is NOT sharded, so each core's softmax
is fully local:  Z[k] = sum_q exp(s[q,k]);  out += exp(s) @ (V/Z)  and the
two k-shards' partial outputs ADD. The host sums the pair, transposes the
channel-major core output, and adds the residual x.

Phase-split design (v2):
  Phase 0 (head): DMA x^T (+ones row) in 8 chunks; project QT [64,512]x8,
    KT [64,512]x4 (k-shard only), V [128,64]x16 through a small PSUM pool.
  Phase 1 (conveyor): per k-tile (128 k values) per q-half (2048):
    4 unpacked score MMs (contract 64) fill a [128,2048] PSUM slot
    (2-slot ring = all 8 banks); the slot is drained by EITHER
      - ScalarE: exp ACTIVATE -> ET bf16 slice + accum Z partial, or
      - VectorE: Schraudolph exp2 (one tensor_scalar: i16(s*K1+K2) whose
        bit pattern IS bf16 exp) + bf16 reduce for the Z partial,
    so the two engines chew score chunks in parallel.
  Phase 2 (dense): AV with V-as-weights: out^T[f,q] accumulated in 8
    one-bank PSUM tiles [64,512] over 16 k-tile matmuls each (N=512,
    one LDW per k-tile). Back-to-back MMs keep the PE busy so the HAM
    clock-gate warms to 2.4 GHz. Evac + DMA out [64,4096] channel-major.

PSUM pools are sequential scopes (qkv -> scores -> acc) so each phase can
use the full 8 banks.
"""

import numpy as np
import ml_dtypes  # noqa: F401  (np bf16 support registered on import)

B, L, C = 4, 4096, 64
NCORES = 8
KSH = L // 2          # k columns per core: 2048
NKT = KSH // 128      # 16 k-tiles per core
NQ5 = L // 512        # 8 q-chunks of 512
NCH = NKT * 2         # 32 score chunks of [128, 2048]

# Schraudolph exp2-in-bf16 constants: exp(x) ~= bitcast_bf16(i16(x*K1 + K2)).
# bf16 = top 16 bits of f32: exponent scale 2^7. K1 = 2^7/ln2. K2 tuned
# numerically (see _tune note) for min rel err (~+-3%) with round-to-nearest.
SCH_K1 = 128.0 / np.log(2.0)
SCH_K2 = 16250.0  # 127*2^7 = 16256 minus bias correction ~6 (tuned offline)

# Which score chunks the DVE drains (rest go to ScalarE). Tune for engine
# balance; DVE also carries evac/Z/gv work. 0 = all-ACT.
DVE_CHUNKS = frozenset()

_cache = {}


def _patch_ldw_opt():
    # walrus is invoked with --enable-ldw-opt=false hardcoded; redundant
    # LDWEIGHTS (same-weight matmul runs) are a large PE-queue cost.
    import concourse.bass_utils as bu
    if getattr(bu, "_ldw_patched", False):
        return
    orig = bu.run_command

    def run_command_ldw(cmd, *a, **kw):
        if isinstance(cmd, list):
            cmd = [c.replace("--enable-ldw-opt=false", "--enable-ldw-opt=true")
                   if isinstance(c, str) else c for c in cmd]
        return orig(cmd, *a, **kw)

    bu.run_command = run_command_ldw
    bu._ldw_patched = True


def _build():
    import concourse.bacc as bacc
    import concourse.mybir as mybir
    import concourse.tile as tile

    bf16 = mybir.dt.bfloat16
    i16 = mybir.dt.int16
    f32 = mybir.dt.float32
    f32r = mybir.dt.float32r
    AF = mybir.ActivationFunctionType
    AX = mybir.AxisListType
    ALU = mybir.AluOpType

    nc = bacc.Bacc("TRN2", target_bir_lowering=False, debug=False)

    xt_d = nc.dram_tensor("xt", [C + 1, L], f32r, kind="ExternalInput")
    wq_d = nc.dram_tensor("wq", [C + 1, C], f32r, kind="ExternalInput")
    wk_d = nc.dram_tensor("wk", [C + 1, C], f32r, kind="ExternalInput")
    wv_d = nc.dram_tensor("wv", [C + 1, C], f32r, kind="ExternalInput")
    o_d = nc.dram_tensor("o", [C, L], f32, kind="ExternalOutput")

    with tile.TileContext(nc) as tc:
        with (
            tc.tile_pool(name="consts", bufs=1) as consts,
            tc.tile_pool(name="sb", bufs=1) as sb,
            tc.tile_pool(name="obp", bufs=2) as obp,
        ):
            # ---------- head: DMAs + projections ----------
            wq_s = consts.tile([C + 1, C], f32r)
            wk_s = consts.tile([C + 1, C], f32r)
            wv_s = consts.tile([C + 1, C], f32r)
            nc.sync.dma_start(out=wq_s, in_=wq_d.ap())
            nc.sync.dma_start(out=wk_s, in_=wk_d.ap())
            nc.sync.dma_start(out=wv_s, in_=wv_d.ap())

            xt_c = []
            for c in range(NQ5):
                t = sb.tile([C + 1, 512], f32r, tag=f"xt{c}")
                nc.sync.dma_start(out=t, in_=xt_d.ap()[:, c * 512:(c + 1) * 512])
                xt_c.append(t)

            # SBUF tiles that outlive phase boundaries
            qt_c = [sb.tile([64, 512], f32r, tag=f"qt{c}", name=f"qt{c}") for c in range(NQ5)]
            kt_c = [sb.tile([64, 512], f32r, tag=f"kt{c}", name=f"kt{c}") for c in range(4)]
            v_ts = [sb.tile([128, C], bf16, tag=f"v{k}", name=f"v{k}") for k in range(NKT)]
            et_ts = [sb.tile([128, L], bf16, tag=f"et{k}", name=f"et{k}") for k in range(NKT)]
            zp = sb.tile([128, NCH], f32)
            z_all = sb.tile([128, NKT], f32)
            rz_all = sb.tile([128, NKT], f32)
            gv_ts = [sb.tile([128, C], bf16, tag=f"gv{k}", name=f"gv{k}") for k in range(NKT)]

            with tc.tile_pool(name="qkvp", bufs=4, space="PSUM") as qkvp:
                for c in range(NQ5):
                    ps = qkvp.tile([128, 512], f32, tag="p")
                    nc.tensor.matmul(ps[0:64, :], lhsT=wq_s[:, 0:64],
                                     rhs=xt_c[c], start=True, stop=True)
                    nc.vector.tensor_copy(out=qt_c[c], in_=ps[0:64, :])
                # K/V only need the k-shard chunks; which chunks those are
                # depends on the core (host passes shard-local xt too?) --
                # no: each core's k-shard is a column range of ITS OWN xt.
                # The shard half is baked per-core via xk chunk indices
                # (same xt tiles; host picks the half by passing `xkoff`).
                # Simplest: compute K/V for BOTH halves is wasteful; instead
                # the host passes xt already rolled so the k-shard is always
                # chunks 0..3. See _in_maps: xt columns are rotated per core.
                for c in range(4):
                    ps = qkvp.tile([128, 512], f32, tag="p")
                    nc.tensor.matmul(ps[0:64, :], lhsT=wk_s[:, 0:64],
                                     rhs=xt_c[c], start=True, stop=True)
                    nc.vector.tensor_copy(out=kt_c[c], in_=ps[0:64, :])
                for kt in range(NKT):
                    ps = qkvp.tile([128, C], f32, tag="v")
                    nc.tensor.matmul(
                        ps,
                        lhsT=xt_c[kt // 4][:, (kt % 4) * 128:(kt % 4 + 1) * 128],
                        rhs=wv_s[:, 0:64], start=True, stop=True,
                    )
                    nc.vector.tensor_copy(out=v_ts[kt], in_=ps)

            # ---------- phase 1: scores + exp conveyor ----------
            with tc.tile_pool(name="scp", bufs=2, space="PSUM") as scp:
                for kt in range(NKT):
                    lk = kt_c[kt // 4][:, (kt % 4) * 128:(kt % 4 + 1) * 128]
                    for qh in range(2):
                        S = scp.tile([128, 2048], f32, tag="s")
                        for c5 in range(4):
                            nc.tensor.matmul(
                                S[:, c5 * 512:(c5 + 1) * 512],
                                lhsT=lk, rhs=qt_c[qh * 4 + c5],
                                start=True, stop=True,
                            )
                        ci = kt * 2 + qh
                        eslice = et_ts[kt][:, qh * 2048:(qh + 1) * 2048]
                        if ci in DVE_CHUNKS:
                            nc.vector.tensor_scalar(
                                out=eslice.bitcast(i16), in0=S,
                                scalar1=float(SCH_K1), scalar2=float(SCH_K2),
                                op0=ALU.mult, op1=ALU.add,
                            )
                            nc.vector.reduce_sum(
                                out=zp[:, ci:ci + 1], in_=eslice, axis=AX.X,
                            )
                        else:
                            nc.scalar.activation(
                                out=eslice, in_=S, func=AF.Exp,
                                accum_out=zp[:, ci:ci + 1],
                            )

            # Z -> 1/Z -> gv (all tiny, batched at the phase boundary)
            zv = zp.rearrange("p (k h) -> p k h", h=2)
            nc.vector.reduce_sum(out=z_all, in_=zv, axis=AX.X)
            nc.vector.reciprocal(out=rz_all, in_=z_all)
            for kt in range(NKT):
                nc.vector.tensor_scalar_mul(
                    gv_ts[kt], v_ts[kt], rz_all[:, kt:kt + 1]
                )

            # ---------- phase 2: dense AV (V as weights, out^T[f, q]) ----------
            o_ap = o_d.ap()
            with tc.tile_pool(name="accp", bufs=1, space="PSUM") as accp:
                acc = [accp.tile([128, 512], f32, tag=f"a{j}", name=f"a{j}")
                       for j in range(NQ5)]
                for kt in range(NKT):
                    for j in range(NQ5):
                        nc.tensor.matmul(
                            acc[j][0:64, :],
                            lhsT=gv_ts[kt],
                            rhs=et_ts[kt][:, j * 512:(j + 1) * 512],
                            start=(kt == 0), stop=(kt == NKT - 1),
                            skip_group_check=True,
                        )
                for j in range(NQ5):
                    ob = obp.tile([64, 512], f32, tag="ob")
                    nc.vector.tensor_copy(out=ob, in_=acc[j][0:64, :])
                    nc.sync.dma_start(
                        out=o_ap[:, j * 512:(j + 1) * 512], in_=ob,
                    )

    nc.compile()
    return nc


def _get_nc():
    # NOTE: do NOT patch --enable-ldw-opt=true: walrus codegen crashes in
    # visitInstLdweights (and the one NEFF that did compile crashed the NC).
    if "nc" not in _cache:
        _cache["nc"] = _build()
    return _cache["nc"]


def _in_maps(x, Wq, bq, Wk, bk, Wv, bv):
    s = 1.0 / np.sqrt(np.float32(C))
    wq1 = (np.concatenate([Wq, bq[None, :]], 0) * s).astype(np.float32)
    wk1 = np.concatenate([Wk, bk[None, :]], 0).astype(np.float32)
    wv1 = np.concatenate([Wv, bv[None, :]], 0).astype(np.float32)
    maps = []
    for core in range(NCORES):
        b, half = core // 2, core % 2
        x1t = np.ascontiguousarray(np.concatenate(
            [x[b], np.ones((L, 1), np.float32)], 1
        ).T.astype(np.float32))              # [65, L]
        # Roll so this core's k-shard sits in columns [0, KSH): the kernel
        # always takes k from chunks 0..3 and q from all 8 chunks.
        if half == 1:
            x1t = np.ascontiguousarray(np.roll(x1t, -KSH, axis=1))
        maps.append({"xt": x1t, "wq": wq1, "wk": wk1, "wv": wv1})
    return maps


def _unshard(outs, x):
    full = np.empty((B, L, C), np.float32)
    for b in range(B):
        o0 = outs[2 * b].astype(np.float32)       # [C, L] in rolled coords
        o1 = outs[2 * b + 1].astype(np.float32)   # rolled by -KSH
        o1 = np.roll(o1, KSH, axis=1)
        full[b] = (o0 + o1).T + x[b]
    return full


def _run(x, Wq, bq, Wk, bk, Wv, bv, trace=False):
    from concourse.bass_utils import run_bass_kernel_spmd

    nc = _get_nc()
    maps = _in_maps(x, Wq, bq, Wk, bk, Wv, bv)
    res = run_bass_kernel_spmd(
        nc, maps, core_ids=list(range(NCORES)), trace=trace
    )
    outs = [r["o"] for r in res.results]
    return _unshard(outs, x), res


def kernel(x, Wq, bq, Wk, bk, Wv, bv):
    x = np.asarray(x, np.float32)
    full, _ = _run(
        x,
        np.asarray(Wq, np.float32), np.asarray(bq, np.float32),
        np.asarray(Wk, np.float32), np.asarray(bk, np.float32),
        np.asarray(Wv, np.float32), np.asarray(bv, np.float32),
    )
    return full


# revision 6
# speedup vs baseline: 1.2335x; 1.1005x over previous
"""Trainium2 Bass kernel for nn_Attention1D (B=4, L=4096, C=64).

reference:
    Q = x@Wq + bq ; K = x@Wk + bk ; V = x@Wv + bv          (per batch b)
    s = Q @ K.T / sqrt(C)                                   [L_q, L_k]
    attn = softmax(s, axis=q)      # normalize over QUERY axis
    out = attn @ V + x

Sharding: 8 cores = 4 batches x 2 key-shards. softmax normalizes over q
(not sharded) so each core's softmax is local: Z[k] = sum_q exp(s[q,k]),
out += exp(s) @ (V/Z); the two k-shards' partial outputs ADD on the host,
which also transposes the channel-major core output and adds residual x.
Core (b, 1) gets x^T rolled by -2048 so its k-shard is always chunks 0-3;
the host unrolls its output.

Phase-split design (PSUM pools are sequential scopes so each phase gets
all 8 banks):
  Head: one combined-weights DMA + x^T in 8 chunks; early dummy ACTIVATE
    pre-loads the exp table; K/Q/V projections (contract 65 = bias row);
    Q evacs on ScalarE, K/V evacs on VectorE (parallel chains).
  Phase 1 conveyor: per (k-tile, q-half): 4 unpacked score MMs (contract
    64, N=512, f32r full rate) fill a [128,2048] PSUM slot (2-slot ring);
    slots drain in parallel on two engines:
      ScalarE: exp ACTIVATE -> ET bf16 + accum Z partial   (~2.25us)
      VectorE: Schraudolph exp: i16(s*K1+K2) IS bf16 exp2 bits (one
        tensor_scalar), then bf16 reduce for Z            (~3.7us)
    A 14-MM dummy burst at the end keeps the PE busy through the phase
    boundary so the HAM clock-gate enters phase 2 warm (2.4 GHz).
  Phase 2: AV with V-as-weights: outT[f,q] in 8 one-bank PSUM tiles
    [64,512]; j-outer/k-inner so each tile finishes early and its
    evac + DMA overlap the remaining MMs. Dense N=512 MMs run at the
    warm 216ns back-to-back rate.
"""

import numpy as np
import ml_dtypes  # noqa: F401  (np bf16 support registered on import)

B, L, C = 4, 4096, 64
NCORES = 8
KSH = L // 2          # k columns per core: 2048
NKT = KSH // 128      # 16 k-tiles per core
NQ5 = L // 512        # 8 q-chunks of 512
NCH = NKT * 2         # 32 score chunks of [128, 2048]

# Schraudolph exp in bf16: exp(x) ~= bitcast_bf16(i16(x*K1 + K2)).
# K1 = 2^7/ln2; K2 calibrated numerically (max rel err ~3.4%, robust to
# round/floor int conversion).
SCH_K1 = 128.0 / np.log(2.0)
SCH_K2 = 16250.75

# Score chunks drained by VectorE (Schraudolph); rest go to ScalarE exp.
DVE_CHUNKS = frozenset(ci for ci in range(4, 30) if ci % 4 == 1)

_cache = {}


def _build():
    import concourse.bacc as bacc
    import concourse.mybir as mybir
    import concourse.tile as tile

    bf16 = mybir.dt.bfloat16
    i16 = mybir.dt.int16
    f32 = mybir.dt.float32
    f32r = mybir.dt.float32r
    AF = mybir.ActivationFunctionType
    AX = mybir.AxisListType
    ALU = mybir.AluOpType

    nc = bacc.Bacc("TRN2", target_bir_lowering=False, debug=False)

    xt_d = nc.dram_tensor("xt", [C + 1, L], f32r, kind="ExternalInput")
    w_d = nc.dram_tensor("w", [C + 1, 3 * C], f32r, kind="ExternalInput")
    o_d = nc.dram_tensor("o", [C, L], f32, kind="ExternalOutput")

    with tile.TileContext(nc) as tc:
        with (
            tc.tile_pool(name="consts", bufs=1) as consts,
            tc.tile_pool(name="sb", bufs=1) as sb,
            tc.tile_pool(name="obp", bufs=2) as obp,
        ):
            # early exp-table preload: tiny ACTIVATE on a zeroed scratch
            scr = consts.tile([128, 8], f32)
            nc.vector.memset(scr, 0.0)
            nc.scalar.activation(out=scr, in_=scr, func=AF.Exp)

            wu = consts.tile([128, 512], bf16)   # warm-burst operand
            nc.vector.memset(wu, 0.0)

            w_s = consts.tile([C + 1, 3 * C], f32r)
            nc.sync.dma_start(out=w_s, in_=w_d.ap())
            wq_s = w_s[:, 0:C]
            wk_s = w_s[:, C:2 * C]
            wv_s = w_s[:, 2 * C:3 * C]

            xt_c = []
            for c in range(NQ5):
                t = sb.tile([C + 1, 512], f32r, tag=f"xt{c}")
                nc.sync.dma_start(out=t, in_=xt_d.ap()[:, c * 512:(c + 1) * 512])
                xt_c.append(t)

            qt_c = [sb.tile([64, 512], f32r, tag=f"qt{c}", name=f"qt{c}")
                    for c in range(NQ5)]
            kt_c = [sb.tile([64, 512], f32r, tag=f"kt{c}", name=f"kt{c}")
                    for c in range(4)]
            v_ts = [sb.tile([128, C], bf16, tag=f"v{k}", name=f"v{k}")
                    for k in range(NKT)]
            et_ts = [sb.tile([128, L], bf16, tag=f"et{k}", name=f"et{k}")
                     for k in range(NKT)]
            zp = sb.tile([128, NCH], f32)
            z_all = sb.tile([128, NKT], f32)
            rz_all = sb.tile([128, NKT], f32)
            gv_ts = [sb.tile([128, C], bf16, tag=f"gv{k}", name=f"gv{k}")
                     for k in range(NKT)]

            with tc.tile_pool(name="qkvp", bufs=4, space="PSUM") as qkvp:
                # K and Q (chunks 0-3) interleaved as x^T chunks land
                for c in range(4):
                    pk = qkvp.tile([128, 512], f32, tag="p", name=f"pk{c}")
                    nc.tensor.matmul(pk[0:64, :], lhsT=wk_s, rhs=xt_c[c],
                                     start=True, stop=True)
                    nc.vector.tensor_copy(out=kt_c[c], in_=pk[0:64, :])
                    pq = qkvp.tile([128, 512], f32, tag="p", name=f"pq{c}")
                    nc.tensor.matmul(pq[0:64, :], lhsT=wq_s, rhs=xt_c[c],
                                     start=True, stop=True)
                    nc.scalar.copy(out=qt_c[c], in_=pq[0:64, :])
                # V from the k-shard chunks 0-3 (ready early)
                for kt in range(NKT):
                    pv = qkvp.tile([128, C], f32, tag="v", name=f"pv{kt}")
                    nc.tensor.matmul(
                        pv,
                        lhsT=xt_c[kt // 4][:, (kt % 4) * 128:(kt % 4 + 1) * 128],
                        rhs=wv_s, start=True, stop=True,
                    )
                    nc.vector.tensor_copy(out=v_ts[kt], in_=pv)
                # Q chunks 4-7 (gated by the DMA tail)
                for c in range(4, NQ5):
                    pq = qkvp.tile([128, 512], f32, tag="p", name=f"pq{c}")
                    nc.tensor.matmul(pq[0:64, :], lhsT=wq_s, rhs=xt_c[c],
                                     start=True, stop=True)
                    nc.scalar.copy(out=qt_c[c], in_=pq[0:64, :])

            # ---------- phase 1: scores + exp conveyor ----------
            with tc.tile_pool(name="scp", bufs=2, space="PSUM") as scp:
                for kt in range(NKT):
                    lk = kt_c[kt // 4][:, (kt % 4) * 128:(kt % 4 + 1) * 128]
                    for qh in range(2):
                        S = scp.tile([128, 2048], f32, tag="s")
                        for c5 in range(4):
                            nc.tensor.matmul(
                                S[:, c5 * 512:(c5 + 1) * 512],
                                lhsT=lk, rhs=qt_c[qh * 4 + c5],
                                start=True, stop=True,
                            )
                        ci = kt * 2 + qh
                        eslice = et_ts[kt][:, qh * 2048:(qh + 1) * 2048]
                        if ci in DVE_CHUNKS:
                            nc.vector.tensor_scalar(
                                out=eslice.bitcast(i16), in0=S,
                                scalar1=float(SCH_K1), scalar2=float(SCH_K2),
                                op0=ALU.mult, op1=ALU.add,
                            )
                            nc.vector.reduce_sum(
                                out=zp[:, ci:ci + 1], in_=eslice, axis=AX.X,
                            )
                        else:
                            nc.scalar.activation(
                                out=eslice, in_=S, func=AF.Exp,
                                accum_out=zp[:, ci:ci + 1],
                            )
                    if kt == 13:
                        # early Z/rz/gv for k-tiles 0..13: off the phase-2
                        # critical path (their zp partials are complete)
                        zv0 = zp[:, 0:28].rearrange("p (k h) -> p k h", h=2)
                        nc.vector.reduce_sum(out=z_all[:, 0:14], in_=zv0,
                                             axis=AX.X)
                        nc.vector.reciprocal(out=rz_all[:, 0:14],
                                             in_=z_all[:, 0:14])
                        for k2 in range(14):
                            nc.vector.tensor_scalar_mul(
                                gv_ts[k2], v_ts[k2], rz_all[:, k2:k2 + 1]
                            )
                # keep the PE busy through the phase boundary (HAM warm)
                Sw = scp.tile([128, 2048], f32, tag="s")
                for i in range(14):
                    nc.tensor.matmul(
                        Sw[:, (i % 4) * 512:(i % 4 + 1) * 512],
                        lhsT=wu[:, 0:128], rhs=wu,
                        start=True, stop=True,
                    )

            zv1 = zp[:, 28:32].rearrange("p (k h) -> p k h", h=2)
            nc.vector.reduce_sum(out=z_all[:, 14:16], in_=zv1, axis=AX.X)
            nc.vector.reciprocal(out=rz_all[:, 14:16], in_=z_all[:, 14:16])
            for k2 in range(14, NKT):
                nc.vector.tensor_scalar_mul(
                    gv_ts[k2], v_ts[k2], rz_all[:, k2:k2 + 1]
                )

            # ---------- phase 2: dense AV (V as weights, outT[f, q]) ----------
            o_ap = o_d.ap()
            with tc.tile_pool(name="accp", bufs=1, space="PSUM") as accp:
                acc = [accp.tile([128, 512], f32, tag=f"a{j}", name=f"a{j}")
                       for j in range(NQ5)]
                for j in range(NQ5):
                    for kt in range(NKT):
                        nc.tensor.matmul(
                            acc[j][0:64, :],
                            lhsT=gv_ts[kt],
                            rhs=et_ts[kt][:, j * 512:(j + 1) * 512],
                            start=(kt == 0), stop=(kt == NKT - 1),
                            skip_group_check=True,
                        )
                    ob = obp.tile([64, 512], f32, tag="ob")
                    nc.vector.tensor_copy(out=ob, in_=acc[j][0:64, :])
                    nc.sync.dma_start(
                        out=o_ap[:, j * 512:(j + 1) * 512], in_=ob,
                    )

    nc.compile()
    return nc


def _get_nc():
    if "nc" not in _cache:
        _cache["nc"] = _build()
    return _cache["nc"]


def _in_maps(x, Wq, bq, Wk, bk, Wv, bv):
    s = 1.0 / np.sqrt(np.float32(C))
    wq1 = (np.concatenate([Wq, bq[None, :]], 0) * s).astype(np.float32)
    wk1 = np.concatenate([Wk, bk[None, :]], 0).astype(np.float32)
    wv1 = np.concatenate([Wv, bv[None, :]], 0).astype(np.float32)
    w = np.ascontiguousarray(np.concatenate([wq1, wk1, wv1], 1))
    maps = []
    for core in range(NCORES):
        b, half = core // 2, core % 2
        x1t = np.ascontiguousarray(np.concatenate(
            [x[b], np.ones((L, 1), np.float32)], 1
        ).T.astype(np.float32))              # [65, L]
        # Roll so this core's k-shard sits in columns [0, KSH): the kernel
        # always takes k from chunks 0..3 and q from all 8 chunks.
        if half == 1:
            x1t = np.ascontiguousarray(np.roll(x1t, -KSH, axis=1))
        maps.append({"xt": x1t, "w": w})
    return maps


def _unshard(outs, x):
    full = np.empty((B, L, C), np.float32)
    for b in range(B):
        o0 = outs[2 * b].astype(np.float32)       # [C, L]
        o1 = outs[2 * b + 1].astype(np.float32)   # [C, L] rolled by -KSH
        o1 = np.roll(o1, KSH, axis=1)
        full[b] = (o0 + o1).T + x[b]
    return full


def _run(x, Wq, bq, Wk, bk, Wv, bv, trace=False):
    from concourse.bass_utils import run_bass_kernel_spmd

    nc = _get_nc()
    maps = _in_maps(x, Wq, bq, Wk, bk, Wv, bv)
    res = run_bass_kernel_spmd(
        nc, maps, core_ids=list(range(NCORES)), trace=trace
    )
    outs = [r["o"] for r in res.results]
    return _unshard(outs, x), res


def kernel(x, Wq, bq, Wk, bk, Wv, bv):
    x = np.asarray(x, np.float32)
    full, _ = _run(
        x,
        np.asarray(Wq, np.float32), np.asarray(bq, np.float32),
        np.asarray(Wk, np.float32), np.asarray(bk, np.float32),
        np.asarray(Wv, np.float32), np.asarray(bv, np.float32),
    )
    return full
